# revision 10
# baseline (speedup 1.0000x reference)
"""Two-layer GAT (8-head 2->128, then 1-head 128->4 + log_softmax) on 8 TRN2 cores.

Strategy: destination-node sharding with degree-sorted 128-row ELL tiles.
Per-edge source features are fetched with bulk `dma_gather`
(InstDMAGatherAnt, mlp Q7 library): one call gathers a block of slot-columns
(~8k edges) at ~2ns/idx of GPSIMD time vs ~1us per 128-edge indirect DMA.

dma_gather constraints engineered around:
  - int16 indices -> the flat [N,16]-f32 node table is viewed as [N/4, 64]
    (256B rows, 4 nodes each): idx = node//4 <= 25088. The wanted 64B
    sub-entry is extracted on DVE with 4 host-precomputed one-hot masks.
  - elem_size must be a 256B multiple -> 256B/edge HBM traffic.
  - <=64 descriptors/packet -> single_packet=False.
  - SWDGE ring capacity -> calls of <=~9k idx on 4 rotating queues,
    dynamic_dma_scratch_size=32768.

Blocks pad their <=4 tiles to a uniform slot width (degree sort makes the
waste ~3%) so ALL per-edge vector math runs as one instruction per block
(DVE instruction dispatch overhead dominates otherwise).

Tables (f32): layer 1: X[n]  = [x0 x1 | 0*14]               (x-only; a_src1
              computed on DVE from the rank-2 structure a_src1 = x @ As)
              layer 2: T2[q] = [h3(4) | a_src2 | junk*11]   (q = core-major)
Layer-1 padding slots carry an all-zero mask (x -> 0) plus a -1e30 bias in
the 5th mask column added to e so exp -> 0. Layer-2 padding slots hit a
dummy table entry with a_src2 = -1e30. Junk table columns are never
extracted. Segment-softmax max-subtraction skipped (value ranges small).
Layer-1 aggregation uses the rank-2 structure of h1 = x @ W1: only sums of
alpha*x (2 cols) are reduced per dst, then expanded through W1 with one PE
matmul per block. An AllGather shares the T2 table between layers.
"""

import os
import numpy as np
from contextlib import ExitStack

import concourse.bass as bass
import concourse.bacc as bacc
import concourse.tile as tile
from concourse import mybir
from concourse.bass import AP
from concourse.bass_utils import run_bass_kernel_spmd

P = 128
NCORE = 8
NEG = 0.2
EPS = 1e-16
NEGBIG = -1.0e30
F32 = mybir.dt.float32
I16 = mybir.dt.int16

# consts column map
AS0X, AS1X, AD0, AD1 = 0, 16, 32, 40
W1BLK, W2EXT, B2, B1, IDENT = 48, 176, 182, 186, 187
CW = 320

BLK_COLS = 72          # max padded slot-columns per dma_gather call
BLK_TILES = 4          # max tiles per block (PSUM: 4*128 <= 512 f32)


def _v(t_ap: AP, off: int, dims) -> AP:
    """View with t_ap's partition dim and custom free dims [[step,count],...]."""
    return AP(t_ap.tensor, t_ap.offset + off, [list(t_ap.ap[0])] + [list(d) for d in dims])


def _dv(handle, off: int, dims) -> AP:
    """DRAM view with custom dims."""
    base = handle[:]
    return AP(base.tensor, off, [list(d) for d in dims])


def _plan(src: np.ndarray, dst: np.ndarray, N: int):
    """Host-side index-only preprocessing: degree sort, ELL tiling with
    block-uniform widths, gather index/mask arrays."""
    E = src.shape[0]
    deg = np.bincount(dst, minlength=N).astype(np.int64)
    T = int(np.ceil(N / (P * NCORE)))          # local tiles per core
    NT = T * NCORE
    N_pad = NT * P
    assert N_pad % 4 == 0
    order = np.concatenate([np.argsort(-deg, kind="stable"), np.arange(N, N_pad)])
    deg_pad = np.concatenate([deg, np.zeros(N_pad - N, np.int64)])
    odeg = deg_pad[order]
    tile_max = odeg.reshape(NT, P).max(axis=1)           # [NT] global tiles
    D_i = np.maximum(tile_max.reshape(T, NCORE).max(axis=1), 1)  # [T]

    # blocks of <=BLK_TILES consecutive tiles padded to the block max width
    # (D_i is non-increasing, so Dblk = D of the first tile)
    blocks = []  # (g0, ng, Dblk, coff)
    colbase = np.zeros(T, np.int64)
    g0 = 0
    off = 0
    while g0 < T:
        Dblk = int(D_i[g0])
        ng = 1
        while (g0 + ng < T and ng < BLK_TILES
               and (ng + 1) * Dblk <= BLK_COLS):
            ng += 1
        ng = min(ng, T - g0)
        for gg in range(ng):
            colbase[g0 + gg] = off + gg * Dblk
        blocks.append((g0, ng, Dblk, off))
        off += ng * Dblk
        g0 += ng
    S = off

    inv_order = np.empty(N_pad, np.int64)
    inv_order[order] = np.arange(N_pad)

    # pos2: row of node n in the (chunked) allgathered T2 table (core-major)
    q = np.arange(N_pad)
    g = q // P
    pos_of_q = (g % NCORE) * (T * P) + (g // NCORE) * P + (q % P)
    pos2 = np.empty(N_pad, np.int64)
    pos2[order[q]] = pos_of_q

    # edges sorted by dst
    eorder = np.argsort(dst, kind="stable")
    dsts = dst[eorder]
    srcs = src[eorder]
    csr = np.zeros(N + 1, np.int64)
    csr[1:] = np.cumsum(deg)
    j = np.arange(E) - csr[dsts]              # rank within dst segment
    qe = inv_order[dsts]
    ge = qe // P
    de = qe % P
    ce = ge % NCORE
    ie = ge // NCORE
    cole = colbase[ie] + j

    # per-slot source node id (layer1) / table position (layer2); dummy N_pad
    sid1 = np.full((NCORE, P, S), N_pad, np.int64)
    sid2 = np.full((NCORE, P, S), N_pad, np.int64)
    pad1 = np.ones((NCORE, P, S), bool)
    sid1[ce, de, cole] = srcs
    pad1[ce, de, cole] = False
    sid2[ce, de, cole] = pos2[srcs]

    def pack_idx(sid):
        """idx blob [NCORE, P, S*8] i16: wrapped-16, replicated across the 8
        partition groups, slot-column-major g = w*128 + d."""
        row = (sid // 4).astype(np.int16)          # [C, P, S]
        idxb = np.zeros((NCORE, P, S * 8), np.int16)
        for c in range(NCORE):
            flat = row[c].T.reshape(S * P)          # g = w*128 + d major
            wrap = flat.reshape(S * 8, 16).T        # [16, S*8]
            for grp in range(8):
                idxb[c, grp * 16:(grp + 1) * 16, :] = wrap
        return idxb

    def pack_msk(sid, ncolm, padbig=None):
        sub = (sid % 4).astype(np.int64)
        mskb = np.zeros((NCORE, P, S, ncolm), np.float32)
        cc, dd, ww = np.meshgrid(np.arange(NCORE), np.arange(P), np.arange(S),
                                 indexing="ij")
        mskb[cc, dd, ww, sub] = 1.0
        if padbig is not None:
            mskb[..., :4][padbig] = 0.0
            mskb[..., 4] = np.where(padbig, NEGBIG, 0.0)
        return mskb.reshape(NCORE, P, S * ncolm)

    idx1 = pack_idx(sid1)
    msk1 = pack_msk(sid1, 5, pad1)
    idx2 = pack_idx(sid2)
    msk2 = pack_msk(sid2, 4)

    # dst node ids per (core, partition, local tile)
    og = order.reshape(NT, P)                  # [g, d]
    dstid = np.empty((NCORE, P, T), np.int64)
    for c in range(NCORE):
        dstid[c] = og[c::NCORE].transpose(1, 0)  # [P, T]

    return dict(E=E, T=T, N_pad=N_pad, S=S, blocks=blocks,
                order=order, pos_of_q=pos_of_q, dstid=dstid,
                idx1=idx1, msk1=msk1, idx2=idx2, msk2=msk2)


def _consts(W1, att_src1, att_dst1, b1, W2, att_src2, att_dst2, b2):
    W1r = W1.reshape(2, 8, 16)
    As = np.einsum("khc,hc->kh", W1r, att_src1)    # [2, 8]
    Ad = np.einsum("khc,hc->kh", W1r, att_dst1)
    c = np.zeros((P, CW), np.float32)
    c[:, AS0X:AS0X + 8] = As[0]
    c[:, AS1X:AS1X + 8] = As[1]
    c[:, AD0:AD0 + 8] = Ad[0]
    c[:, AD1:AD1 + 8] = Ad[1]
    w1blk = np.zeros((16, 128), np.float32)
    for k in range(2):
        for h in range(8):
            w1blk[k * 8 + h, h * 16:(h + 1) * 16] = W1r[k, h]
    c[:16, W1BLK:W1BLK + 128] = w1blk
    c[:, W2EXT:W2EXT + 4] = W2
    c[:, W2EXT + 4] = W2 @ att_src2[0]
    c[:, W2EXT + 5] = W2 @ att_dst2[0]
    c[:, B2:B2 + 4] = b2
    c[:, B1] = b1
    c[:, IDENT:IDENT + 128] = np.eye(P, dtype=np.float32)
    # dummy table rows (gathered by padding slots; must be finite)
    dum1 = np.zeros((1, 64), np.float32)
    dum2 = np.zeros((1, 64), np.float32)
    dum2[0, 4] = NEGBIG         # T2 dummy sub 0: h3 = 0, a_src2 = -BIG
    return c, dum1, dum2


def _build(T, S, blocks, N_pad, use_prelu=True):
    WC = max(BLK_COLS, max(ng * Dblk for (_g, ng, Dblk, _c) in blocks))
    R1 = N_pad // 4 + 1         # gather-table rows incl. dummy
    nc = bacc.Bacc("TRN2", target_bir_lowering=False,
                   num_swdge_queues=4, dynamic_dma_scratch_size=32768)
    xin = nc.declare_dram_parameter("xpad", [N_pad, 2], F32, isOutput=False)
    idx1in = nc.declare_dram_parameter("idx1", [P, S * 8], I16, isOutput=False)
    msk1in = nc.declare_dram_parameter("msk1", [P, S * 5], F32, isOutput=False)
    idx2in = nc.declare_dram_parameter("idx2", [P, S * 8], I16, isOutput=False)
    msk2in = nc.declare_dram_parameter("msk2", [P, S * 4], F32, isOutput=False)
    xdin = nc.declare_dram_parameter("xd", [P, T * 2], F32, isOutput=False)
    cin = nc.declare_dram_parameter("consts", [P, CW], F32, isOutput=False)
    d1in = nc.declare_dram_parameter("dum1", [1, 64], F32, isOutput=False)
    d2in = nc.declare_dram_parameter("dum2", [1, 64], F32, isOutput=False)
    oext = nc.declare_dram_parameter("out", [T * P, 4], F32, isOutput=True)

    xtab = nc.dram_tensor("xtab", [R1, 64], F32)
    t2tab = nc.dram_tensor("t2tab", [R1, 64], F32, addr_space="Shared")
    z2sh = nc.dram_tensor("z2sh", [T * P, 16], F32)

    J = N_pad // P          # nodes per partition in the table build
    ACT = mybir.ActivationFunctionType
    ALU = mybir.AluOpType

    with tile.TileContext(nc) as tc, ExitStack() as ctx:
        persist = ctx.enter_context(tc.tile_pool(name="persist", bufs=1))
        build = ctx.enter_context(tc.tile_pool(name="build", bufs=1))
        iop = ctx.enter_context(tc.tile_pool(name="iop", bufs=4))
        gp = ctx.enter_context(tc.tile_pool(name="gath", bufs=3))
        zp = ctx.enter_context(tc.tile_pool(name="zp", bufs=2))
        wk = ctx.enter_context(tc.tile_pool(name="work", bufs=2))
        sm = ctx.enter_context(tc.tile_pool(name="small", bufs=3))
        pp = ctx.enter_context(tc.tile_pool(name="psA", bufs=2, space="PSUM"))
        pq = ctx.enter_context(tc.tile_pool(name="psB", bufs=2, space="PSUM"))

        csb = persist.tile([P, CW], F32)
        nc.sync.dma_start(out=csb[:], in_=cin[:])
        h3eS = persist.tile([P, T * 16], F32)
        nc.vector.memset(h3eS[:], 0.0)
        x_sb = persist.tile([P, J * 2], F32)
        nc.sync.dma_start(out=x_sb[:], in_=xin[:].rearrange("(p j) c -> p (j c)", p=P))
        adstE = persist.tile([P, T * 8], F32)
        nc.sync.dma_start(out=xtab[R1 - 1:R1, :], in_=d1in[:])
        nc.sync.dma_start(out=t2tab[R1 - 1:R1, :], in_=d2in[:])
        tc.strict_bb_all_engine_barrier()

        # ---- prologue: write X table (x in cols 0:2 of each 16-f32 entry;
        # node = p*J + j), chunked; other cols are never extracted ----
        nch = max(1, (J + 97) // 98)
        jc = (J + nch - 1) // nch
        for c0 in range(0, J, jc):
            jn = min(jc, J - c0)
            z1_sb = build.tile([P, jc * 16], F32, tag="zb")
            nc.vector.memset(z1_sb[:], 0.0)
            nc.vector.tensor_copy(
                out=_v(z1_sb[:], 0, [[16, jn], [1, 2]]),
                in_=_v(x_sb[:], c0 * 2, [[2, jn], [1, 2]]))
            nc.sync.dma_start(
                out=_dv(xtab, c0 * 16, [[J * 16, P], [1, jn * 16]]),
                in_=z1_sb[:, 0:jn * 16])

        # a_dst per (partition, tile, head) from this core's dst-shard x rows
        xd = persist.tile([P, T * 2], F32)
        nc.sync.dma_start(out=xd[:], in_=xdin[:])
        ttd = build.tile([P, T * 8], F32, tag="td")
        nc.vector.tensor_tensor(
            out=adstE[:].rearrange("p (t h) -> p t h", h=8),
            in0=_v(xd[:], 0, [[2, T], [0, 8]]), in1=_v(csb[:], AD0, [[0, T], [1, 8]]),
            op=ALU.mult)
        nc.vector.tensor_tensor(
            out=ttd[:].rearrange("p (t h) -> p t h", h=8),
            in0=_v(xd[:], 1, [[2, T], [0, 8]]), in1=_v(csb[:], AD1, [[0, T], [1, 8]]),
            op=ALU.mult)
        nc.vector.tensor_tensor(out=adstE[:], in0=adstE[:], in1=ttd[:], op=ALU.add)

        tc.strict_bb_all_engine_barrier()

        def lrelu_exp(dst_t, src_t, n):
            if use_prelu:
                tmp = wk.tile([P, n], F32, tag="lrtmp")
                nc.scalar.activation(out=tmp[:], in_=src_t, func=ACT.Prelu, alpha=NEG)
                nc.scalar.activation(out=dst_t, in_=tmp[:], func=ACT.Exp)
            else:
                tmp = wk.tile([P, n], F32, tag="lrtmp")
                nc.vector.tensor_scalar_mul(tmp[:], src_t, NEG)
                nc.vector.tensor_tensor(out=tmp[:], in0=src_t, in1=tmp[:], op=ALU.max)
                nc.scalar.activation(out=dst_t, in_=tmp[:], func=ACT.Exp)

        def gather_extract(bi, b, tab, idxin, mskin, nval, nmsk):
            """Gather block b's W slot-columns from tab and mask-extract nval
            f32 per edge -> (zblk [P, W*nval], mskt)."""
            (g0, ng, Dblk, coff) = b
            W = ng * Dblk
            idxt = iop.tile([P, WC * 8], I16, tag="idx")
            nc.sync.dma_start(out=idxt[:, 0:W * 8], in_=idxin[:, coff * 8:(coff + W) * 8])
            mskt = iop.tile([P, WC * 5], F32, tag="msk")
            nc.sync.dma_start(out=mskt[:, 0:W * nmsk],
                              in_=mskin[:, coff * nmsk:(coff + W) * nmsk])
            raw = gp.tile([P, WC * 64], F32, tag="raw")
            nc.gpsimd.dma_gather(
                raw[:, 0:W * 64].rearrange("p (w e) -> p w e", e=64),
                tab[:],
                idxt[:, 0:W * 8],
                W * P, W * P, 64,
                queue_num=bi % 4, single_packet=False)
            zblk = zp.tile([P, WC * 10], F32, tag="z")
            tq = wk.tile([P, 3 * WC * 5], F32, tag="tq")
            outs = [_v(zblk[:], 0, [[nval, W], [1, nval]])] + [
                _v(tq[:], o * WC * 5, [[nval, W], [1, nval]]) for o in range(3)]
            for o in range(4):
                nc.vector.tensor_tensor(
                    out=outs[o],
                    in0=_v(raw[:], o * 16, [[64, W], [1, nval]]),
                    in1=_v(mskt[:], o, [[nmsk, W], [0, nval]]),
                    op=ALU.mult)
            nc.vector.tensor_tensor(out=outs[1], in0=outs[1], in1=outs[2], op=ALU.add)
            nc.vector.tensor_tensor(out=outs[1], in0=outs[1], in1=outs[3], op=ALU.add)
            nc.vector.tensor_tensor(out=outs[0], in0=outs[0], in1=outs[1], op=ALU.add)
            return zblk, mskt

        # ---- layer 1 ----
        for bi, b in enumerate(blocks):
            (g0, ng, Dblk, coff) = b
            W = ng * Dblk
            xe, mskt = gather_extract(bi, b, xtab, idx1in, msk1in, 2, 5)
            # e[t,h,s] = x0*As0[h] + x1*As1[h] + a_dst[t,h] + padbig
            nW = 8 * W
            e = wk.tile([P, nW], F32, tag="e1")
            tb = wk.tile([P, nW], F32, tag="tb1")
            ev = _v(e[:], 0, [[8 * Dblk, ng], [Dblk, 8], [1, Dblk]])
            tv = _v(tb[:], 0, [[8 * Dblk, ng], [Dblk, 8], [1, Dblk]])
            nc.vector.tensor_tensor(
                out=ev,
                in0=_v(xe[:], 0, [[2 * Dblk, ng], [0, 8], [2, Dblk]]),
                in1=_v(csb[:], AS0X, [[0, ng], [1, 8], [0, Dblk]]),
                op=ALU.mult)
            nc.vector.tensor_tensor(
                out=tv,
                in0=_v(xe[:], 1, [[2 * Dblk, ng], [0, 8], [2, Dblk]]),
                in1=_v(csb[:], AS1X, [[0, ng], [1, 8], [0, Dblk]]),
                op=ALU.mult)
            nc.vector.tensor_tensor(
                out=tv, in0=tv,
                in1=_v(adstE[:], g0 * 8, [[8, ng], [1, 8], [0, Dblk]]),
                op=ALU.add)
            nc.vector.tensor_tensor(
                out=tv, in0=tv,
                in1=_v(mskt[:], 4, [[5 * Dblk, ng], [0, 8], [5, Dblk]]),
                op=ALU.add)
            nc.vector.tensor_tensor(out=e[:], in0=e[:], in1=tb[:], op=ALU.add)
            ex = wk.tile([P, nW], F32, tag="ex1")
            lrelu_exp(ex[:], e[:], nW)
            s = sm.tile([P, 8 * BLK_TILES], F32, tag="s1")
            nc.vector.tensor_reduce(
                out=s[:, 0:8 * ng], in_=ex[:, 0:nW].rearrange("p (a j) -> p a j", j=Dblk),
                axis=mybir.AxisListType.X, op=ALU.add)
            rs = sm.tile([P, 8 * BLK_TILES], F32, tag="rs1")
            nc.vector.tensor_scalar_add(rs[:, 0:8 * ng], s[:, 0:8 * ng], EPS)
            nc.vector.reciprocal(rs[:, 0:8 * ng], rs[:, 0:8 * ng])
            prod = wk.tile([P, 2 * nW], F32, tag="pr1")
            for k in range(2):
                nc.vector.tensor_tensor(
                    out=_v(prod[:], k * nW, [[8 * Dblk, ng], [Dblk, 8], [1, Dblk]]),
                    in0=_v(ex[:], 0, [[8 * Dblk, ng], [Dblk, 8], [1, Dblk]]),
                    in1=_v(xe[:], k, [[2 * Dblk, ng], [0, 8], [2, Dblk]]),
                    op=ALU.mult)
            G = sm.tile([P, 16 * BLK_TILES], F32, tag="G1")
            nc.vector.tensor_reduce(
                out=G[:, 0:16 * ng],
                in_=prod[:, 0:2 * nW].rearrange("p (a j) -> p a j", j=Dblk),
                axis=mybir.AxisListType.X, op=ALU.add)
            # G layout: [k, t, h] (k slowest); normalize by rs[t, h] into
            # tile-major Gn [t][k][h] so each tile's 16 cols are contiguous
            Gn = sm.tile([P, 16 * BLK_TILES], F32, tag="Gn1")
            nc.vector.tensor_tensor(
                out=_v(Gn[:], 0, [[16, ng], [8, 2], [1, 8]]),
                in0=_v(G[:], 0, [[8, ng], [8 * ng, 2], [1, 8]]),
                in1=_v(rs[:], 0, [[8, ng], [0, 2], [1, 8]]),
                op=ALU.mult)
            GnT = sm.tile([16, BLK_TILES * P], F32, tag="GnT")
            for gg in range(ng):
                pt = pp.tile([P, P], F32, tag="pt")
                nc.tensor.transpose(
                    out=pt[0:16, :],
                    in_=Gn[:, gg * 16:(gg + 1) * 16],
                    identity=csb[:, IDENT:IDENT + 128])
                nc.scalar.copy(out=GnT[0:16, gg * P:(gg + 1) * P], in_=pt[0:16, :])
            o1p = pq.tile([P, BLK_TILES * P], F32, tag="o1p")
            nc.tensor.matmul(
                out=o1p[:, 0:ng * P],
                lhsT=csb[0:16, W1BLK:W1BLK + 128],
                rhs=GnT[0:16, 0:ng * P],
                start=True, stop=True)
            h2T = wk.tile([P, BLK_TILES * P], F32, tag="h2T")
            nc.scalar.activation(
                out=h2T[:, 0:ng * P], in_=o1p[:, 0:ng * P],
                func=ACT.Relu, bias=csb[:, B1:B1 + 1], scale=1.0)
            h3p = pq.tile([P, BLK_TILES * 8], F32, tag="h3p")
            for gg in range(ng):
                nc.tensor.matmul(
                    out=h3p[:, gg * 8:gg * 8 + 6],
                    lhsT=h2T[:, gg * P:(gg + 1) * P],
                    rhs=csb[:, W2EXT:W2EXT + 6],
                    start=True, stop=True)
            nc.vector.tensor_copy(
                out=_v(h3eS[:], g0 * 16, [[16, ng], [1, 6]]),
                in_=_v(h3p[:], 0, [[8, ng], [1, 6]]))

        # ---- share T2 ----
        nc.sync.dma_start(
            out=_dv(z2sh, 0, [[16, P], [P * 16, T], [1, 16]]),
            in_=h3eS[:].rearrange("p (t c) -> p t c", c=16))
        tc.strict_bb_all_engine_barrier()
        nc.gpsimd.collective_compute(
            "AllGather", ALU.bypass,
            replica_groups=[list(range(NCORE))],
            ins=[z2sh[:]], outs=[t2tab[0:R1 - 1, :]])
        tc.strict_bb_all_engine_barrier()

        # ---- layer 2 ----
        for bi, b in enumerate(blocks):
            (g0, ng, Dblk, coff) = b
            W = ng * Dblk
            z2blk, _m = gather_extract(bi, b, t2tab, idx2in, msk2in, 5, 4)
            e2 = wk.tile([P, W], F32, tag="e2")
            nc.vector.tensor_tensor(
                out=_v(e2[:], 0, [[Dblk, ng], [1, Dblk]]),
                in0=_v(z2blk[:], 4, [[5 * Dblk, ng], [5, Dblk]]),
                in1=_v(h3eS[:], g0 * 16 + 5, [[16, ng], [0, Dblk]]),
                op=ALU.add)
            ex2 = wk.tile([P, W], F32, tag="ex2")
            lrelu_exp(ex2[:], e2[:], W)
            s2 = sm.tile([P, BLK_TILES], F32, tag="s2")
            nc.vector.tensor_reduce(
                out=s2[:, 0:ng], in_=ex2[:, 0:W].rearrange("p (a j) -> p a j", j=Dblk),
                axis=mybir.AxisListType.X, op=ALU.add)
            rs2 = sm.tile([P, BLK_TILES], F32, tag="rs2")
            nc.vector.tensor_scalar_add(rs2[:, 0:ng], s2[:, 0:ng], EPS)
            nc.vector.reciprocal(rs2[:, 0:ng], rs2[:, 0:ng])
            prod2 = wk.tile([P, 4 * W], F32, tag="pr2")
            nc.vector.tensor_tensor(
                out=_v(prod2[:], 0, [[4 * Dblk, ng], [Dblk, 4], [1, Dblk]]),
                in0=_v(ex2[:], 0, [[Dblk, ng], [0, 4], [1, Dblk]]),
                in1=_v(z2blk[:], 0, [[5 * Dblk, ng], [1, 4], [5, Dblk]]),
                op=ALU.mult)
            M2 = sm.tile([P, 4 * BLK_TILES], F32, tag="M2")
            nc.vector.tensor_reduce(
                out=M2[:, 0:4 * ng],
                in_=prod2[:, 0:4 * W].rearrange("p (a j) -> p a j", j=Dblk),
                axis=mybir.AxisListType.X, op=ALU.add)
            o2 = sm.tile([P, 4 * BLK_TILES], F32, tag="o2")
            nc.vector.tensor_tensor(
                out=_v(o2[:], 0, [[4, ng], [1, 4]]),
                in0=_v(M2[:], 0, [[4, ng], [1, 4]]),
                in1=_v(rs2[:], 0, [[1, ng], [0, 4]]),
                op=ALU.mult)
            nc.vector.tensor_tensor(
                out=_v(o2[:], 0, [[4, ng], [1, 4]]),
                in0=_v(o2[:], 0, [[4, ng], [1, 4]]),
                in1=_v(csb[:], B2, [[0, ng], [1, 4]]),
                op=ALU.add)
            # log_softmax over c
            mx = sm.tile([P, BLK_TILES], F32, tag="mx")
            nc.vector.tensor_reduce(
                out=mx[:, 0:ng], in_=o2[:, 0:4 * ng].rearrange("p (t c) -> p t c", c=4),
                axis=mybir.AxisListType.X, op=ALU.max)
            z = sm.tile([P, 4 * BLK_TILES], F32, tag="z")
            nc.vector.tensor_tensor(
                out=_v(z[:], 0, [[4, ng], [1, 4]]),
                in0=_v(o2[:], 0, [[4, ng], [1, 4]]),
                in1=_v(mx[:], 0, [[1, ng], [0, 4]]),
                op=ALU.subtract)
            ez = sm.tile([P, 4 * BLK_TILES], F32, tag="ez")
            nc.scalar.activation(out=ez[:, 0:4 * ng], in_=z[:, 0:4 * ng], func=ACT.Exp)
            se = sm.tile([P, BLK_TILES], F32, tag="se")
            nc.vector.tensor_reduce(
                out=se[:, 0:ng], in_=ez[:, 0:4 * ng].rearrange("p (t c) -> p t c", c=4),
                axis=mybir.AxisListType.X, op=ALU.add)
            lse = sm.tile([P, BLK_TILES], F32, tag="lse")
            nc.scalar.activation(out=lse[:, 0:ng], in_=se[:, 0:ng], func=ACT.Ln)
            res = sm.tile([P, 4 * BLK_TILES], F32, tag="res")
            nc.vector.tensor_tensor(
                out=_v(res[:], 0, [[4, ng], [1, 4]]),
                in0=_v(z[:], 0, [[4, ng], [1, 4]]),
                in1=_v(lse[:], 0, [[1, ng], [0, 4]]),
                op=ALU.subtract)
            nc.sync.dma_start(
                out=_dv(oext, g0 * P * 4, [[4, P], [P * 4, ng], [1, 4]]),
                in_=_v(res[:], 0, [[4, ng], [1, 4]]))

    nc.compile()
    return nc


def kernel(**inputs) -> np.ndarray:
    x = np.asarray(inputs["x"], np.float32)
    edge_index = np.asarray(inputs["edge_index"])
    N = x.shape[0]
    src = edge_index[0].astype(np.int64)
    dst = edge_index[1].astype(np.int64)

    plan = _plan(src, dst, N)
    T, S, N_pad = plan["T"], plan["S"], plan["N_pad"]

    consts, dum1, dum2 = _consts(
        np.asarray(inputs["W1"], np.float32), np.asarray(inputs["att_src1"], np.float32),
        np.asarray(inputs["att_dst1"], np.float32), np.asarray(inputs["b1"], np.float32),
        np.asarray(inputs["W2"], np.float32), np.asarray(inputs["att_src2"], np.float32),
        np.asarray(inputs["att_dst2"], np.float32), np.asarray(inputs["b2"], np.float32))

    xpad = np.zeros((N_pad, 2), np.float32)
    xpad[:N] = x

    use_prelu = os.environ.get("GAT_NO_PRELU", "0") != "1"
    nc = _build(T, S, plan["blocks"], N_pad, use_prelu=use_prelu)

    in_maps = []
    for c in range(NCORE):
        in_maps.append({
            "xpad": xpad,
            "idx1": plan["idx1"][c],
            "msk1": plan["msk1"][c],
            "idx2": plan["idx2"][c],
            "msk2": plan["msk2"][c],
            "xd": xpad[plan["dstid"][c]].reshape(P, -1),
            "consts": consts,
            "dum1": dum1,
            "dum2": dum2,
        })

    if os.environ.get("GAT_SIM", "0") == "1":
        from concourse.bass_interp import MultiCoreSim
        sim = MultiCoreSim(nc, NCORE)
        for c in range(NCORE):
            for k, v in in_maps[c].items():
                sim.cores[c].tensor(k)[:] = v
        sim.simulate()
        outs = [np.array(sim.cores[c].tensor("out")[:]) for c in range(NCORE)]
    else:
        trace = os.environ.get("GAT_TRACE", "0") == "1"
        res = run_bass_kernel_spmd(nc, in_maps, list(range(NCORE)), trace=trace)
        if trace:
            print(f"HW exec time: {res.exec_time_ns} ns")
        outs = [res.results[c]["out"] for c in range(NCORE)]

    big = np.concatenate(outs, axis=0)          # [NCORE*T*P, 4] core-major
    full = np.empty((N_pad, 4), np.float32)
    q = np.arange(N_pad)
    full[plan["order"][q]] = big[plan["pos_of_q"][q]]
    return full[:N]


# revision 11
# speedup vs baseline: 1.2030x; 1.2030x over previous
"""Two-layer GAT (8-head 2->128, then 1-head 128->4 + log_softmax) on 8 TRN2 cores.

Strategy: destination-node sharding with degree-sorted 128-row ELL tiles.
Per-edge source features are fetched with bulk `dma_gather`
(InstDMAGatherAnt, mlp Q7 library): one call gathers a block of slot-columns
(~8k edges) at ~2ns/idx of GPSIMD time vs ~1us per 128-edge indirect DMA.

dma_gather constraints engineered around:
  - int16 indices -> the flat [N,16]-f32 node table is viewed as [N/4, 64]
    (256B rows, 4 nodes each): idx = node//4 <= 25088. The wanted 64B
    sub-entry is extracted on DVE with 4 host-precomputed one-hot masks.
  - elem_size must be a 256B multiple -> 256B/edge HBM traffic.
  - <=64 descriptors/packet -> single_packet=False.
  - SWDGE ring capacity -> calls of <=~9k idx on 4 rotating queues,
    dynamic_dma_scratch_size=32768.

Blocks pad their <=4 tiles to a uniform slot width (degree sort makes the
waste ~3%) so ALL per-edge vector math runs as one instruction per block
(DVE instruction dispatch overhead dominates otherwise).

Tables (f32): layer 1: X[n]  = [x0 x1 | 0*14]               (x-only; a_src1
              computed on DVE from the rank-2 structure a_src1 = x @ As)
              layer 2: T2[q] = [h3(4) | a_src2 | junk*11]   (q = core-major)
Layer-1 padding slots carry an all-zero mask (x -> 0) plus a -1e30 bias in
the 5th mask column added to e so exp -> 0. Layer-2 padding slots hit a
dummy table entry with a_src2 = -1e30. Junk table columns are never
extracted. Segment-softmax max-subtraction skipped (value ranges small).
Layer-1 aggregation uses the rank-2 structure of h1 = x @ W1: only sums of
alpha*x (2 cols) are reduced per dst, then expanded through W1 with one PE
matmul per block. An AllGather shares the T2 table between layers.
"""

import os
import numpy as np
from contextlib import ExitStack

import concourse.bass as bass
import concourse.bacc as bacc
import concourse.tile as tile
from concourse import mybir
from concourse.bass import AP
from concourse.bass_utils import run_bass_kernel_spmd

P = 128
NCORE = 8
NEG = 0.2
EPS = 1e-16
NEGBIG = -1.0e30
F32 = mybir.dt.float32
I16 = mybir.dt.int16

# consts column map
AS0X, AS1X, AD0, AD1 = 0, 16, 32, 40
W1BLK, W2EXT, B2, B1, IDENT = 48, 176, 182, 186, 187
CW = 320

BLK_COLS = 72          # max padded slot-columns per dma_gather call
BLK_TILES = 4          # max tiles per block (PSUM: 4*128 <= 512 f32)


def _v(t_ap: AP, off: int, dims) -> AP:
    """View with t_ap's partition dim and custom free dims [[step,count],...]."""
    return AP(t_ap.tensor, t_ap.offset + off, [list(t_ap.ap[0])] + [list(d) for d in dims])


def _dv(handle, off: int, dims) -> AP:
    """DRAM view with custom dims."""
    base = handle[:]
    return AP(base.tensor, off, [list(d) for d in dims])


def _plan(src: np.ndarray, dst: np.ndarray, N: int):
    """Host-side index-only preprocessing: degree sort, ELL tiling with
    block-uniform widths, gather index/mask arrays."""
    E = src.shape[0]
    deg = np.bincount(dst, minlength=N).astype(np.int64)
    T = int(np.ceil(N / (P * NCORE)))          # local tiles per core
    NT = T * NCORE
    N_pad = NT * P
    assert N_pad % 4 == 0
    order = np.concatenate([np.argsort(-deg, kind="stable"), np.arange(N, N_pad)])
    deg_pad = np.concatenate([deg, np.zeros(N_pad - N, np.int64)])
    odeg = deg_pad[order]
    tile_max = odeg.reshape(NT, P).max(axis=1)           # [NT] global tiles
    D_i = np.maximum(tile_max.reshape(T, NCORE).max(axis=1), 1)  # [T]

    # blocks of <=BLK_TILES consecutive tiles padded to the block max width
    # (D_i is non-increasing, so Dblk = D of the first tile)
    blocks = []  # (g0, ng, Dblk, coff)
    colbase = np.zeros(T, np.int64)
    g0 = 0
    off = 0
    while g0 < T:
        Dblk = int(D_i[g0])
        ng = 1
        while (g0 + ng < T and ng < BLK_TILES
               and (ng + 1) * Dblk <= BLK_COLS):
            ng += 1
        ng = min(ng, T - g0)
        for gg in range(ng):
            colbase[g0 + gg] = off + gg * Dblk
        blocks.append((g0, ng, Dblk, off))
        off += ng * Dblk
        g0 += ng
    S = off

    inv_order = np.empty(N_pad, np.int64)
    inv_order[order] = np.arange(N_pad)

    # pos2: row of node n in the (chunked) allgathered T2 table (core-major)
    q = np.arange(N_pad)
    g = q // P
    pos_of_q = (g % NCORE) * (T * P) + (g // NCORE) * P + (q % P)
    pos2 = np.empty(N_pad, np.int64)
    pos2[order[q]] = pos_of_q

    # edges sorted by dst
    eorder = np.argsort(dst, kind="stable")
    dsts = dst[eorder]
    srcs = src[eorder]
    csr = np.zeros(N + 1, np.int64)
    csr[1:] = np.cumsum(deg)
    j = np.arange(E) - csr[dsts]              # rank within dst segment
    qe = inv_order[dsts]
    ge = qe // P
    de = qe % P
    ce = ge % NCORE
    ie = ge // NCORE
    cole = colbase[ie] + j

    # per-slot source node id (layer1) / table position (layer2); dummy N_pad
    sid1 = np.full((NCORE, P, S), N_pad, np.int64)
    sid2 = np.full((NCORE, P, S), N_pad, np.int64)
    pad1 = np.ones((NCORE, P, S), bool)
    sid1[ce, de, cole] = srcs
    pad1[ce, de, cole] = False
    sid2[ce, de, cole] = pos2[srcs]

    def pack_idx(sid):
        """idx blob [NCORE, P, S*8] i16: wrapped-16, replicated across the 8
        partition groups, slot-column-major g = w*128 + d."""
        row = (sid // 4).astype(np.int16)          # [C, P, S]
        idxb = np.zeros((NCORE, P, S * 8), np.int16)
        for c in range(NCORE):
            flat = row[c].T.reshape(S * P)          # g = w*128 + d major
            wrap = flat.reshape(S * 8, 16).T        # [16, S*8]
            for grp in range(8):
                idxb[c, grp * 16:(grp + 1) * 16, :] = wrap
        return idxb

    def pack_msk(sid, ncolm, padbig=None):
        sub = (sid % 4).astype(np.int64)
        mskb = np.zeros((NCORE, P, S, ncolm), np.float32)
        cc, dd, ww = np.meshgrid(np.arange(NCORE), np.arange(P), np.arange(S),
                                 indexing="ij")
        mskb[cc, dd, ww, sub] = 1.0
        if padbig is not None:
            mskb[..., :4][padbig] = 0.0
            mskb[..., 4] = np.where(padbig, NEGBIG, 0.0)
        return mskb.reshape(NCORE, P, S * ncolm)

    idx1 = pack_idx(sid1)
    msk1 = pack_msk(sid1, 5, pad1)
    idx2 = pack_idx(sid2)
    msk2 = pack_msk(sid2, 4)

    # dst node ids per (core, partition, local tile)
    og = order.reshape(NT, P)                  # [g, d]
    dstid = np.empty((NCORE, P, T), np.int64)
    for c in range(NCORE):
        dstid[c] = og[c::NCORE].transpose(1, 0)  # [P, T]

    return dict(E=E, T=T, N_pad=N_pad, S=S, blocks=blocks,
                order=order, pos_of_q=pos_of_q, dstid=dstid,
                idx1=idx1, msk1=msk1, idx2=idx2, msk2=msk2)


def _consts(W1, att_src1, att_dst1, b1, W2, att_src2, att_dst2, b2):
    W1r = W1.reshape(2, 8, 16)
    As = np.einsum("khc,hc->kh", W1r, att_src1)    # [2, 8]
    Ad = np.einsum("khc,hc->kh", W1r, att_dst1)
    c = np.zeros((P, CW), np.float32)
    c[:, AS0X:AS0X + 8] = As[0]
    c[:, AS1X:AS1X + 8] = As[1]
    c[:, AD0:AD0 + 8] = Ad[0]
    c[:, AD1:AD1 + 8] = Ad[1]
    w1blk = np.zeros((16, 128), np.float32)
    for k in range(2):
        for h in range(8):
            w1blk[k * 8 + h, h * 16:(h + 1) * 16] = W1r[k, h]
    c[:16, W1BLK:W1BLK + 128] = w1blk
    c[:, W2EXT:W2EXT + 4] = W2
    c[:, W2EXT + 4] = W2 @ att_src2[0]
    c[:, W2EXT + 5] = W2 @ att_dst2[0]
    c[:, B2:B2 + 4] = b2
    c[:, B1] = b1
    c[:, IDENT:IDENT + 128] = np.eye(P, dtype=np.float32)
    # dummy table rows (gathered by padding slots; must be finite)
    dum1 = np.zeros((1, 64), np.float32)
    dum2 = np.zeros((1, 64), np.float32)
    dum2[0, 4] = NEGBIG         # T2 dummy sub 0: h3 = 0, a_src2 = -BIG
    return c, dum1, dum2


def _build(T, S, blocks, N_pad, use_prelu=True):
    WC = max(BLK_COLS, max(ng * Dblk for (_g, ng, Dblk, _c) in blocks))
    R1 = N_pad // 4 + 1         # gather-table rows incl. dummy
    nc = bacc.Bacc("TRN2", target_bir_lowering=False,
                   num_swdge_queues=4, dynamic_dma_scratch_size=32768)
    xin = nc.declare_dram_parameter("xpad", [N_pad, 2], F32, isOutput=False)
    idx1in = nc.declare_dram_parameter("idx1", [P, S * 8], I16, isOutput=False)
    msk1in = nc.declare_dram_parameter("msk1", [P, S * 5], F32, isOutput=False)
    idx2in = nc.declare_dram_parameter("idx2", [P, S * 8], I16, isOutput=False)
    msk2in = nc.declare_dram_parameter("msk2", [P, S * 4], F32, isOutput=False)
    xdin = nc.declare_dram_parameter("xd", [P, T * 2], F32, isOutput=False)
    cin = nc.declare_dram_parameter("consts", [P, CW], F32, isOutput=False)
    d1in = nc.declare_dram_parameter("dum1", [1, 64], F32, isOutput=False)
    d2in = nc.declare_dram_parameter("dum2", [1, 64], F32, isOutput=False)
    oext = nc.declare_dram_parameter("out", [T * P, 4], F32, isOutput=True)

    xtab = nc.dram_tensor("xtab", [R1, 64], F32)
    t2tab = nc.dram_tensor("t2tab", [R1, 64], F32, addr_space="Shared")
    z2sh = nc.dram_tensor("z2sh", [T * P, 16], F32)

    J = N_pad // P          # nodes per partition in the table build
    ACT = mybir.ActivationFunctionType
    ALU = mybir.AluOpType

    with tile.TileContext(nc) as tc, ExitStack() as ctx:
        persist = ctx.enter_context(tc.tile_pool(name="persist", bufs=1))
        build = ctx.enter_context(tc.tile_pool(name="build", bufs=1))
        iop = ctx.enter_context(tc.tile_pool(name="iop", bufs=5))
        gp = ctx.enter_context(tc.tile_pool(name="gath", bufs=4))
        zp = ctx.enter_context(tc.tile_pool(name="zp", bufs=3))
        tqp = ctx.enter_context(tc.tile_pool(name="tqp", bufs=2))
        wk = ctx.enter_context(tc.tile_pool(name="work", bufs=2))
        sm = ctx.enter_context(tc.tile_pool(name="small", bufs=3))
        pp = ctx.enter_context(tc.tile_pool(name="psA", bufs=2, space="PSUM"))
        pq = ctx.enter_context(tc.tile_pool(name="psB", bufs=2, space="PSUM"))

        csb = persist.tile([P, CW], F32)
        nc.sync.dma_start(out=csb[:], in_=cin[:])
        h3eS = persist.tile([P, T * 16], F32)
        nc.vector.memset(h3eS[:], 0.0)
        x_sb = persist.tile([P, J * 2], F32)
        nc.sync.dma_start(out=x_sb[:], in_=xin[:].rearrange("(p j) c -> p (j c)", p=P))
        adstE = persist.tile([P, T * 8], F32)
        nc.sync.dma_start(out=xtab[R1 - 1:R1, :], in_=d1in[:])
        nc.sync.dma_start(out=t2tab[R1 - 1:R1, :], in_=d2in[:])
        tc.strict_bb_all_engine_barrier()

        # ---- prologue: write X table (x in cols 0:2 of each 16-f32 entry;
        # node = p*J + j), chunked; other cols are never extracted ----
        nch = max(1, (J + 97) // 98)
        jc = (J + nch - 1) // nch
        for c0 in range(0, J, jc):
            jn = min(jc, J - c0)
            z1_sb = build.tile([P, jc * 16], F32, tag="zb")
            nc.vector.memset(z1_sb[:], 0.0)
            nc.vector.tensor_copy(
                out=_v(z1_sb[:], 0, [[16, jn], [1, 2]]),
                in_=_v(x_sb[:], c0 * 2, [[2, jn], [1, 2]]))
            nc.sync.dma_start(
                out=_dv(xtab, c0 * 16, [[J * 16, P], [1, jn * 16]]),
                in_=z1_sb[:, 0:jn * 16])

        # a_dst per (partition, tile, head) from this core's dst-shard x rows
        xd = persist.tile([P, T * 2], F32)
        nc.sync.dma_start(out=xd[:], in_=xdin[:])
        ttd = build.tile([P, T * 8], F32, tag="td")
        nc.vector.tensor_tensor(
            out=adstE[:].rearrange("p (t h) -> p t h", h=8),
            in0=_v(xd[:], 0, [[2, T], [0, 8]]), in1=_v(csb[:], AD0, [[0, T], [1, 8]]),
            op=ALU.mult)
        nc.vector.tensor_tensor(
            out=ttd[:].rearrange("p (t h) -> p t h", h=8),
            in0=_v(xd[:], 1, [[2, T], [0, 8]]), in1=_v(csb[:], AD1, [[0, T], [1, 8]]),
            op=ALU.mult)
        nc.vector.tensor_tensor(out=adstE[:], in0=adstE[:], in1=ttd[:], op=ALU.add)

        tc.strict_bb_all_engine_barrier()

        def lrelu_exp(dst_t, src_t, n):
            if use_prelu:
                tmp = wk.tile([P, n], F32, tag="lrtmp")
                nc.scalar.activation(out=tmp[:], in_=src_t, func=ACT.Prelu, alpha=NEG)
                nc.scalar.activation(out=dst_t, in_=tmp[:], func=ACT.Exp)
            else:
                tmp = wk.tile([P, n], F32, tag="lrtmp")
                nc.vector.tensor_scalar_mul(tmp[:], src_t, NEG)
                nc.vector.tensor_tensor(out=tmp[:], in0=src_t, in1=tmp[:], op=ALU.max)
                nc.scalar.activation(out=dst_t, in_=tmp[:], func=ACT.Exp)

        def gather_extract(bi, b, tab, idxin, mskin, nval, nmsk):
            """Gather block b's W slot-columns from tab and mask-extract nval
            f32 per edge -> (zblk [P, W*nval], mskt)."""
            (g0, ng, Dblk, coff) = b
            W = ng * Dblk
            idxt = iop.tile([P, WC * 8], I16, tag="idx")
            nc.sync.dma_start(out=idxt[:, 0:W * 8], in_=idxin[:, coff * 8:(coff + W) * 8])
            mskt = iop.tile([P, WC * 5], F32, tag="msk")
            nc.sync.dma_start(out=mskt[:, 0:W * nmsk],
                              in_=mskin[:, coff * nmsk:(coff + W) * nmsk])
            raw = gp.tile([P, WC * 64], F32, tag="raw")
            nc.gpsimd.dma_gather(
                raw[:, 0:W * 64].rearrange("p (w e) -> p w e", e=64),
                tab[:],
                idxt[:, 0:W * 8],
                W * P, W * P, 64,
                queue_num=bi % 4, single_packet=False)
            zblk = zp.tile([P, WC * 10], F32, tag="z")
            tq = tqp.tile([P, 3 * WC * 5], F32, tag="tq")
            outs = [_v(zblk[:], 0, [[nval, W], [1, nval]])] + [
                _v(tq[:], o * WC * 5, [[nval, W], [1, nval]]) for o in range(3)]
            for o in range(4):
                nc.vector.tensor_tensor(
                    out=outs[o],
                    in0=_v(raw[:], o * 16, [[64, W], [1, nval]]),
                    in1=_v(mskt[:], o, [[nmsk, W], [0, nval]]),
                    op=ALU.mult)
            nc.vector.tensor_tensor(out=outs[1], in0=outs[1], in1=outs[2], op=ALU.add)
            nc.vector.tensor_tensor(out=outs[1], in0=outs[1], in1=outs[3], op=ALU.add)
            nc.vector.tensor_tensor(out=outs[0], in0=outs[0], in1=outs[1], op=ALU.add)
            return zblk, mskt

        # ---- layer 1 ----
        for bi, b in enumerate(blocks):
            (g0, ng, Dblk, coff) = b
            W = ng * Dblk
            xe, mskt = gather_extract(bi, b, xtab, idx1in, msk1in, 2, 5)
            # e[t,h,s] = x0*As0[h] + x1*As1[h] + a_dst[t,h] + padbig
            nW = 8 * W
            e = wk.tile([P, nW], F32, tag="e1")
            tb = wk.tile([P, nW], F32, tag="tb1")
            ev = _v(e[:], 0, [[8 * Dblk, ng], [Dblk, 8], [1, Dblk]])
            tv = _v(tb[:], 0, [[8 * Dblk, ng], [Dblk, 8], [1, Dblk]])
            nc.vector.tensor_tensor(
                out=ev,
                in0=_v(xe[:], 0, [[2 * Dblk, ng], [0, 8], [2, Dblk]]),
                in1=_v(csb[:], AS0X, [[0, ng], [1, 8], [0, Dblk]]),
                op=ALU.mult)
            nc.vector.tensor_tensor(
                out=tv,
                in0=_v(xe[:], 1, [[2 * Dblk, ng], [0, 8], [2, Dblk]]),
                in1=_v(csb[:], AS1X, [[0, ng], [1, 8], [0, Dblk]]),
                op=ALU.mult)
            nc.vector.tensor_tensor(
                out=tv, in0=tv,
                in1=_v(adstE[:], g0 * 8, [[8, ng], [1, 8], [0, Dblk]]),
                op=ALU.add)
            nc.vector.tensor_tensor(
                out=tv, in0=tv,
                in1=_v(mskt[:], 4, [[5 * Dblk, ng], [0, 8], [5, Dblk]]),
                op=ALU.add)
            nc.vector.tensor_tensor(out=e[:], in0=e[:], in1=tb[:], op=ALU.add)
            ex = wk.tile([P, nW], F32, tag="ex1")
            lrelu_exp(ex[:], e[:], nW)
            s = sm.tile([P, 8 * BLK_TILES], F32, tag="s1")
            nc.vector.tensor_reduce(
                out=s[:, 0:8 * ng], in_=ex[:, 0:nW].rearrange("p (a j) -> p a j", j=Dblk),
                axis=mybir.AxisListType.X, op=ALU.add)
            rs = sm.tile([P, 8 * BLK_TILES], F32, tag="rs1")
            nc.vector.tensor_scalar_add(rs[:, 0:8 * ng], s[:, 0:8 * ng], EPS)
            nc.vector.reciprocal(rs[:, 0:8 * ng], rs[:, 0:8 * ng])
            prod = wk.tile([P, 2 * nW], F32, tag="pr1")
            for k in range(2):
                nc.vector.tensor_tensor(
                    out=_v(prod[:], k * nW, [[8 * Dblk, ng], [Dblk, 8], [1, Dblk]]),
                    in0=_v(ex[:], 0, [[8 * Dblk, ng], [Dblk, 8], [1, Dblk]]),
                    in1=_v(xe[:], k, [[2 * Dblk, ng], [0, 8], [2, Dblk]]),
                    op=ALU.mult)
            G = sm.tile([P, 16 * BLK_TILES], F32, tag="G1")
            nc.vector.tensor_reduce(
                out=G[:, 0:16 * ng],
                in_=prod[:, 0:2 * nW].rearrange("p (a j) -> p a j", j=Dblk),
                axis=mybir.AxisListType.X, op=ALU.add)
            # G layout: [k, t, h] (k slowest); normalize by rs[t, h] into
            # tile-major Gn [t][k][h] so each tile's 16 cols are contiguous
            Gn = sm.tile([P, 16 * BLK_TILES], F32, tag="Gn1")
            nc.vector.tensor_tensor(
                out=_v(Gn[:], 0, [[16, ng], [8, 2], [1, 8]]),
                in0=_v(G[:], 0, [[8, ng], [8 * ng, 2], [1, 8]]),
                in1=_v(rs[:], 0, [[8, ng], [0, 2], [1, 8]]),
                op=ALU.mult)
            GnT = sm.tile([16, BLK_TILES * P], F32, tag="GnT")
            for gg in range(ng):
                pt = pp.tile([P, P], F32, tag="pt")
                nc.tensor.transpose(
                    out=pt[0:16, :],
                    in_=Gn[:, gg * 16:(gg + 1) * 16],
                    identity=csb[:, IDENT:IDENT + 128])
                nc.scalar.copy(out=GnT[0:16, gg * P:(gg + 1) * P], in_=pt[0:16, :])
            o1p = pq.tile([P, BLK_TILES * P], F32, tag="o1p")
            nc.tensor.matmul(
                out=o1p[:, 0:ng * P],
                lhsT=csb[0:16, W1BLK:W1BLK + 128],
                rhs=GnT[0:16, 0:ng * P],
                start=True, stop=True)
            h2T = wk.tile([P, BLK_TILES * P], F32, tag="h2T")
            nc.scalar.activation(
                out=h2T[:, 0:ng * P], in_=o1p[:, 0:ng * P],
                func=ACT.Relu, bias=csb[:, B1:B1 + 1], scale=1.0)
            h3p = pq.tile([P, BLK_TILES * 8], F32, tag="h3p")
            for gg in range(ng):
                nc.tensor.matmul(
                    out=h3p[:, gg * 8:gg * 8 + 6],
                    lhsT=h2T[:, gg * P:(gg + 1) * P],
                    rhs=csb[:, W2EXT:W2EXT + 6],
                    start=True, stop=True)
            nc.vector.tensor_copy(
                out=_v(h3eS[:], g0 * 16, [[16, ng], [1, 6]]),
                in_=_v(h3p[:], 0, [[8, ng], [1, 6]]))

        # ---- share T2 ----
        nc.sync.dma_start(
            out=_dv(z2sh, 0, [[16, P], [P * 16, T], [1, 16]]),
            in_=h3eS[:].rearrange("p (t c) -> p t c", c=16))
        tc.strict_bb_all_engine_barrier()
        nc.gpsimd.collective_compute(
            "AllGather", ALU.bypass,
            replica_groups=[list(range(NCORE))],
            ins=[z2sh[:]], outs=[t2tab[0:R1 - 1, :]])
        tc.strict_bb_all_engine_barrier()

        # ---- layer 2 ----
        for bi, b in enumerate(blocks):
            (g0, ng, Dblk, coff) = b
            W = ng * Dblk
            z2blk, _m = gather_extract(bi, b, t2tab, idx2in, msk2in, 5, 4)
            e2 = wk.tile([P, W], F32, tag="e2")
            nc.vector.tensor_tensor(
                out=_v(e2[:], 0, [[Dblk, ng], [1, Dblk]]),
                in0=_v(z2blk[:], 4, [[5 * Dblk, ng], [5, Dblk]]),
                in1=_v(h3eS[:], g0 * 16 + 5, [[16, ng], [0, Dblk]]),
                op=ALU.add)
            ex2 = wk.tile([P, W], F32, tag="ex2")
            lrelu_exp(ex2[:], e2[:], W)
            s2 = sm.tile([P, BLK_TILES], F32, tag="s2")
            nc.vector.tensor_reduce(
                out=s2[:, 0:ng], in_=ex2[:, 0:W].rearrange("p (a j) -> p a j", j=Dblk),
                axis=mybir.AxisListType.X, op=ALU.add)
            rs2 = sm.tile([P, BLK_TILES], F32, tag="rs2")
            nc.vector.tensor_scalar_add(rs2[:, 0:ng], s2[:, 0:ng], EPS)
            nc.vector.reciprocal(rs2[:, 0:ng], rs2[:, 0:ng])
            prod2 = wk.tile([P, 4 * W], F32, tag="pr2")
            nc.vector.tensor_tensor(
                out=_v(prod2[:], 0, [[4 * Dblk, ng], [Dblk, 4], [1, Dblk]]),
                in0=_v(ex2[:], 0, [[Dblk, ng], [0, 4], [1, Dblk]]),
                in1=_v(z2blk[:], 0, [[5 * Dblk, ng], [1, 4], [5, Dblk]]),
                op=ALU.mult)
            M2 = sm.tile([P, 4 * BLK_TILES], F32, tag="M2")
            nc.vector.tensor_reduce(
                out=M2[:, 0:4 * ng],
                in_=prod2[:, 0:4 * W].rearrange("p (a j) -> p a j", j=Dblk),
                axis=mybir.AxisListType.X, op=ALU.add)
            o2 = sm.tile([P, 4 * BLK_TILES], F32, tag="o2")
            nc.vector.tensor_tensor(
                out=_v(o2[:], 0, [[4, ng], [1, 4]]),
                in0=_v(M2[:], 0, [[4, ng], [1, 4]]),
                in1=_v(rs2[:], 0, [[1, ng], [0, 4]]),
                op=ALU.mult)
            nc.vector.tensor_tensor(
                out=_v(o2[:], 0, [[4, ng], [1, 4]]),
                in0=_v(o2[:], 0, [[4, ng], [1, 4]]),
                in1=_v(csb[:], B2, [[0, ng], [1, 4]]),
                op=ALU.add)
            # log_softmax over c
            mx = sm.tile([P, BLK_TILES], F32, tag="mx")
            nc.vector.tensor_reduce(
                out=mx[:, 0:ng], in_=o2[:, 0:4 * ng].rearrange("p (t c) -> p t c", c=4),
                axis=mybir.AxisListType.X, op=ALU.max)
            z = sm.tile([P, 4 * BLK_TILES], F32, tag="z")
            nc.vector.tensor_tensor(
                out=_v(z[:], 0, [[4, ng], [1, 4]]),
                in0=_v(o2[:], 0, [[4, ng], [1, 4]]),
                in1=_v(mx[:], 0, [[1, ng], [0, 4]]),
                op=ALU.subtract)
            ez = sm.tile([P, 4 * BLK_TILES], F32, tag="ez")
            nc.scalar.activation(out=ez[:, 0:4 * ng], in_=z[:, 0:4 * ng], func=ACT.Exp)
            se = sm.tile([P, BLK_TILES], F32, tag="se")
            nc.vector.tensor_reduce(
                out=se[:, 0:ng], in_=ez[:, 0:4 * ng].rearrange("p (t c) -> p t c", c=4),
                axis=mybir.AxisListType.X, op=ALU.add)
            lse = sm.tile([P, BLK_TILES], F32, tag="lse")
            nc.scalar.activation(out=lse[:, 0:ng], in_=se[:, 0:ng], func=ACT.Ln)
            res = sm.tile([P, 4 * BLK_TILES], F32, tag="res")
            nc.vector.tensor_tensor(
                out=_v(res[:], 0, [[4, ng], [1, 4]]),
                in0=_v(z[:], 0, [[4, ng], [1, 4]]),
                in1=_v(lse[:], 0, [[1, ng], [0, 4]]),
                op=ALU.subtract)
            nc.sync.dma_start(
                out=_dv(oext, g0 * P * 4, [[4, P], [P * 4, ng], [1, 4]]),
                in_=_v(res[:], 0, [[4, ng], [1, 4]]))

    nc.compile()
    return nc


def kernel(**inputs) -> np.ndarray:
    x = np.asarray(inputs["x"], np.float32)
    edge_index = np.asarray(inputs["edge_index"])
    N = x.shape[0]
    src = edge_index[0].astype(np.int64)
    dst = edge_index[1].astype(np.int64)

    plan = _plan(src, dst, N)
    T, S, N_pad = plan["T"], plan["S"], plan["N_pad"]

    consts, dum1, dum2 = _consts(
        np.asarray(inputs["W1"], np.float32), np.asarray(inputs["att_src1"], np.float32),
        np.asarray(inputs["att_dst1"], np.float32), np.asarray(inputs["b1"], np.float32),
        np.asarray(inputs["W2"], np.float32), np.asarray(inputs["att_src2"], np.float32),
        np.asarray(inputs["att_dst2"], np.float32), np.asarray(inputs["b2"], np.float32))

    xpad = np.zeros((N_pad, 2), np.float32)
    xpad[:N] = x

    use_prelu = os.environ.get("GAT_NO_PRELU", "0") != "1"
    nc = _build(T, S, plan["blocks"], N_pad, use_prelu=use_prelu)

    in_maps = []
    for c in range(NCORE):
        in_maps.append({
            "xpad": xpad,
            "idx1": plan["idx1"][c],
            "msk1": plan["msk1"][c],
            "idx2": plan["idx2"][c],
            "msk2": plan["msk2"][c],
            "xd": xpad[plan["dstid"][c]].reshape(P, -1),
            "consts": consts,
            "dum1": dum1,
            "dum2": dum2,
        })

    if os.environ.get("GAT_SIM", "0") == "1":
        from concourse.bass_interp import MultiCoreSim
        sim = MultiCoreSim(nc, NCORE)
        for c in range(NCORE):
            for k, v in in_maps[c].items():
                sim.cores[c].tensor(k)[:] = v
        sim.simulate()
        outs = [np.array(sim.cores[c].tensor("out")[:]) for c in range(NCORE)]
    else:
        trace = os.environ.get("GAT_TRACE", "0") == "1"
        res = run_bass_kernel_spmd(nc, in_maps, list(range(NCORE)), trace=trace)
        if trace:
            print(f"HW exec time: {res.exec_time_ns} ns")
        outs = [res.results[c]["out"] for c in range(NCORE)]

    big = np.concatenate(outs, axis=0)          # [NCORE*T*P, 4] core-major
    full = np.empty((N_pad, 4), np.float32)
    q = np.arange(N_pad)
    full[plan["order"][q]] = big[plan["pos_of_q"][q]]
    return full[:N]
